# revision 40
# baseline (speedup 1.0000x reference)
"""Trainium2 Bass kernel for nn_LogLinearAttention (B=2,T=1024,Dm=1024,H=16,D=64,L=12).

Math (validated numerically in a numpy prototype):
  out = ((S*Mw)@V / rowsum(S*Mw)) @ ow + ob   with S = phi(xQ) phi(xK)^T,
  Mw[i,j] = w[i, lev(i,j)],  lev(i,j) = msb((i+1) XOR j)  (0-based, j<=i).
Softmax over levels cancels in num/den, so w~ = exp(logits) is used raw.
phi(a) = elu(a)+1 = max(a + 1, min(exp(a), 1)).

Per 128-token query block bi:
 * inter (key blocks < bi): Fenwick block segments; per segment a state
   A_seg = K_seg^T @ [V_seg|1]; contribution = scale_col * (Q_bi @ A_seg)
   where scale_col = w~[:, 7+g] with the last row (ti=127) level-remapped.
 * intra (diag block): MwdT[j,i] = COLIND^T @ ((REPLT^T @ w~T) * ROWIND)
   built on the PE with 128 one-hot slots (127 dyadic runs shared across
   blocks + a per-block row-127 slot patched via copy_predicated);
   SmdT = (Kp_bi @ Qp_bi^T) * MwdT;  contribution = SmdT^T @ [V|1].

Sharding: 8 cores, core c owns heads {2c, 2c+1} for both batches
(tensor-parallel projections, head-parallel attention, partial output
projections summed on host).
"""

from contextlib import ExitStack

import numpy as np

import concourse.bass as bass
import concourse.tile as tile
import concourse.mybir as mybir
from concourse import bacc
from concourse.bass_utils import run_bass_kernel_spmd
from concourse.masks import make_identity

F32 = mybir.dt.float32
F32R = mybir.dt.float32r
U8 = mybir.dt.uint8

B, T, DM, H, D, L = 2, 1024, 1024, 16, 64, 12
C = 128            # token block
NB = T // C        # 8
NCORES = 8
NTB = B * T // C   # 16 token blocks over (b, t)
KC = DM // 128     # 8 contraction chunks
WALLN = 540        # per-chunk packed weights: wq 128 | wk 128 | wo1 284

AF = mybir.ActivationFunctionType
ALU = mybir.AluOpType


def _msb(v):
    return v.bit_length() - 1


def _decomp(bi):
    """Fenwick decomposition of block-prefix [0, bi): [(beta, size, g), ...]."""
    segs, start = [], 0
    for g in range(7, -1, -1):
        if (bi >> g) & 1:
            segs.append((start, 1 << g, g))
            start += 1 << g
    return segs


# state-tile layout: leaves P0..P6 at slots 0..6; combined segments:
_COMB = {(0, 2): 7, (0, 4): 8, (4, 2): 9}


def _l127(bi):
    return 7 + _msb((bi + 1) ^ bi)


def _build_slot_consts():
    """Shared COLIND/ROWIND [128,128] and REPLT [12,128] (slot-127 zeroed)."""
    colind = np.zeros((128, C), np.float32)
    rowind = np.zeros((128, C), np.float32)
    replt = np.zeros((L, 128), np.float32)
    i1 = np.arange(1, C + 1)
    slot = 0
    for c in range(7):
        for m in range(1 << (6 - c)):
            rows = (((i1 >> (c + 1)) == m) & (((i1 >> c) & 1) == 1) & (i1 < C))
            rowind[slot, :] = rows.astype(np.float32)
            colind[slot, m * (1 << (c + 1)): m * (1 << (c + 1)) + (1 << c)] = 1.0
            replt[c, slot] = 1.0
            slot += 1
    assert slot == 127
    rowind[127, 127] = 1.0
    colind[127, :] = 1.0
    # replt slot-127 column stays zero; the row-127 value is patched into
    # WROW[127, bi, 127] at runtime via copy_predicated.
    return colind, rowind, replt


def _w_fixups():
    """Row-127 level remaps on w~ for inter scale columns: [(bi, tgt, src)]."""
    fixes = []
    for bi in range(NB):
        for (beta, size, g) in _decomp(bi):
            tgt, src = 7 + g, 7 + _msb((bi + 1) ^ beta)
            if src != tgt:
                fixes.append((bi, tgt, src))
    return fixes


_PROGRAM_CACHE = {}


def _build_program(with_o1_bias: bool):
    nc = bacc.Bacc(trn_type="TRN2", target_bir_lowering=False, debug=False,
                   num_devices=NCORES)

    xT = nc.dram_tensor("xT", [DM, B * T], F32, kind="ExternalInput").ap()
    wall = nc.dram_tensor("wall", [DM, WALLN], F32, kind="ExternalInput").ap()
    # cvr: colind_sh 128 | ow 1024   (fp32r-consumed)
    cvr = nc.dram_tensor("cvr", [128, 1152], F32, kind="ExternalInput").ap()
    # cvf: rowind_sh 128 | qb qb1 kb kb1     (fp32-consumed)
    cvf = nc.dram_tensor("cvf", [128, 1032], F32, kind="ExternalInput").ap()
    replt_d = nc.dram_tensor("replt", [L, 128], F32, kind="ExternalInput").ap()
    m127 = nc.dram_tensor("m127", [128, 1], U8, kind="ExternalInput").ap()
    bias1 = nc.dram_tensor("bias1", [128, 284], F32, kind="ExternalInput").ap()
    out_d = nc.dram_tensor("out", [B * T, DM], F32, kind="ExternalOutput").ap()

    fixes = _w_fixups()

    with tile.TileContext(nc) as tc, ExitStack() as ctx:
        const = ctx.enter_context(tc.tile_pool(name="const", bufs=1))
        big = ctx.enter_context(tc.tile_pool(name="big", bufs=1))
        sm = ctx.enter_context(tc.tile_pool(name="sm", bufs=3))
        smd = ctx.enter_context(tc.tile_pool(name="smd", bufs=3))
        acc = ctx.enter_context(tc.tile_pool(name="acc", bufs=2))

        # ---------- input DMAs: interleave weights and x chunks ----------
        wall_sb = const.tile([128, KC, WALLN], F32R)
        xch = big.tile([128, KC, B * T], F32R)
        for k in range(KC):
            nc.sync.dma_start(out=wall_sb[:, k, :],
                              in_=wall[128 * k:128 * (k + 1), :].bitcast(F32R))
            nc.sync.dma_start(out=xch[:, k, :],
                              in_=xT[128 * k:128 * (k + 1), :].bitcast(F32R))
        cvr_sb = const.tile([128, 1152], F32R)
        cvf_sb = const.tile([128, 1032], F32)
        replt_sb = const.tile([L, 128], F32R)
        m127_sb = const.tile([128, 1], U8)
        nc.sync.dma_start(out=cvr_sb, in_=cvr.bitcast(F32R))
        nc.sync.dma_start(out=cvf_sb, in_=cvf)
        nc.sync.dma_start(out=replt_sb, in_=replt_d.bitcast(F32R))
        nc.sync.dma_start(out=m127_sb, in_=m127)
        if with_o1_bias:
            bias1_sb = const.tile([128, 284], F32)
            nc.sync.dma_start(out=bias1_sb, in_=bias1)
        ident = const.tile([128, 128], F32)
        make_identity(nc, ident)
        colind_sb = cvr_sb[:, 0:128]
        ow_sb = cvr_sb[:, 128:1152]
        rowind_sb = cvf_sb[:, 0:1024]

        QpT = big.tile([128, B * T], F32R)
        KpT = big.tile([128, B * T], F32R)
        Kp1 = big.tile([128, NTB, 128], F32R)
        Vp1 = big.tile([128, NTB, 132], F32R)
        wt = big.tile([128, NTB, 24], F32)
        wtT = [big.tile([L, B * T], F32R, name=f"wtT{h}", tag=f"wtT{h}")
               for h in range(2)]
        attn_a = big.tile([128, NTB, 128], F32)
        attnT = big.tile([128, B * T], F32R)

        # ================= phase 1: projections =================
        with tc.tile_pool(name="psP", bufs=2, space="PSUM") as psP:
            # orientation-2: QpT / KpT (phi applied)
            for (woff, bcol, pcol, dst) in ((0, 1024, 1025, QpT),
                                            (128, 1026, 1027, KpT)):
                for sl in range(4):
                    pt = psP.tile([128, 512], F32, tag="o2", bufs=3)
                    for k in range(KC):
                        nc.tensor.matmul(
                            pt, wall_sb[:, k, woff:woff + 128],
                            xch[:, k, 512 * sl:512 * (sl + 1)],
                            start=(k == 0), stop=(k == KC - 1))
                    et = sm.tile([128, 512], F32, tag="o2exp", bufs=2)
                    nc.scalar.activation(et, pt, AF.Exp,
                                         bias=cvf_sb[:, bcol:bcol + 1])
                    ec = sm.tile([128, 512], F32, tag="o2expc", bufs=2)
                    nc.vector.tensor_scalar(out=ec, in0=et, scalar1=0.0, scalar2=1.0, op0=ALU.add, op1=ALU.min)
                    nc.vector.scalar_tensor_tensor(
                        out=dst[:, 512 * sl:512 * (sl + 1)], in0=pt,
                        scalar=cvf_sb[:, pcol:pcol + 1], in1=ec,
                        op0=ALU.add, op1=ALU.max)
            # orientation-1: Kp1 / Vp1 / w~
            for tb in range(NTB):
                pt = psP.tile([128, 284], F32, tag="o1", bufs=3)
                for k in range(KC):
                    nc.tensor.matmul(
                        pt, xch[:, k, 128 * tb:128 * (tb + 1)],
                        wall_sb[:, k, 256:540],
                        start=(k == 0), stop=(k == KC - 1))
                if with_o1_bias:
                    nc.vector.tensor_add(pt, pt, bias1_sb)
                et = sm.tile([128, 128], F32, tag="o1exp")
                nc.scalar.activation(et, pt[:, 132:260], AF.Exp)
                ec = sm.tile([128, 128], F32, tag="o1expc")
                nc.vector.tensor_scalar(out=ec, in0=et, scalar1=0.0, scalar2=1.0, op0=ALU.add, op1=ALU.min)
                nc.scalar.activation(wt[:, tb, :], pt[:, 260:284], AF.Exp)
                nc.vector.scalar_tensor_tensor(
                    out=Kp1[:, tb, :], in0=pt[:, 132:260], scalar=1.0,
                    in1=ec, op0=ALU.add, op1=ALU.max)
                nc.vector.tensor_copy(Vp1[:, tb, :], pt[:, 0:132])
            v4 = Vp1.rearrange("p b (two ss) -> p b two ss", two=2, ss=66)
            nc.vector.memset(v4[:, :, :, 64:65].bitcast(F32), 1.0)
            # w~ transposes (raw levels), then row-127 fixups
            for tb in range(NTB):
                for h in range(2):
                    ptt = psP.tile([12, 128], F32, tag="wtt", bufs=2)
                    nc.tensor.transpose(ptt, wt[:, tb, 12 * h:12 * h + 12],
                                        ident)
                    nc.vector.tensor_copy(wtT[h][:, 128 * tb:128 * (tb + 1)],
                                          ptt)
            fs = NTB * 24
            for (bi, tgt, srcl) in fixes:
                def _wcols(col):
                    return bass.AP(tensor=wt.tensor,
                                   offset=wt.offset + bi * 24 + col,
                                   ap=[[fs, 128], [NB * 24, 2], [12, 2]])
                mk = bass.AP(tensor=m127_sb.tensor, offset=m127_sb.offset,
                             ap=[[1, 128], [0, 2], [0, 2]])
                nc.vector.copy_predicated(out=_wcols(tgt), mask=mk,
                                          data=_wcols(srcl))

        # ================= phase 2: attention =================
        with tc.tile_pool(name="psA", bufs=2, space="PSUM") as psA:
            for b in range(B):
                # leaf + combined Fenwick states; head h valid at
                # partitions [64h, 64h+64) x cols [66h, 66h+66)
                ST = smd.tile([128, 10, 132], F32R, tag="ST", bufs=2)
                for beta in range(7):
                    blk = b * NB + beta
                    pp = psA.tile([128, 132], F32, tag="np", bufs=3)
                    nc.tensor.matmul(pp, Kp1[:, blk, :], Vp1[:, blk, :],
                                     start=True, stop=True)
                    nc.scalar.copy(ST[:, beta, :], pp)
                nc.vector.tensor_add(ST[:, 7, :], ST[:, 0, :], ST[:, 1, :])
                nc.vector.tensor_add(ST[:, 8, :], ST[:, 7, :], ST[:, 2, :])
                nc.vector.tensor_add(ST[:, 8, :], ST[:, 8, :], ST[:, 3, :])
                nc.vector.tensor_add(ST[:, 9, :], ST[:, 4, :], ST[:, 5, :])
                for h in range(2):
                    hp = slice(64 * h, 64 * (h + 1))
                    vc = slice(66 * h, 66 * (h + 1))
                    tokb = slice(C * b * NB, C * (b + 1) * NB)
                    # batched mask build: WROW/MwdT for all 8 blocks at once
                    wr_ps = psA.tile([128, NB * C], F32, tag="wide", bufs=1)
                    for hf in range(2):
                        nc.tensor.matmul(
                            wr_ps[:, 512 * hf:512 * (hf + 1)], replt_sb,
                            wtT[h][:, C * b * NB + 512 * hf:
                                   C * b * NB + 512 * (hf + 1)],
                            start=True, stop=True, skip_group_check=True)
                    wrow = sm.tile([128, NB, 128], F32R, tag="wrow_sb", bufs=2)
                    nc.vector.tensor_mul(
                        wrow, wr_ps.rearrange("p (nb c) -> p nb c", nb=NB),
                        rowind_sb.rearrange("p (nb c) -> p nb c", nb=NB))
                    for bi in range(NB):
                        blk = b * NB + bi
                        lc = 12 * h + _l127(bi)
                        nc.vector.tensor_mul(
                            wrow[:, bi, 127:128], wt[:, blk, lc:lc + 1],
                            cvf_sb[:, 1028:1029])
                    mw_ps = psA.tile([128, NB * C], F32, tag="wide", bufs=1)
                    wrow_f = wrow.rearrange("p nb c -> p (nb c)")
                    for hf in range(2):
                        nc.tensor.matmul(
                            mw_ps[:, 512 * hf:512 * (hf + 1)], colind_sb,
                            wrow_f[:, 512 * hf:512 * (hf + 1)],
                            start=True, stop=True, skip_group_check=True)
                    mwsb = sm.tile([128, NB, 128], F32R, tag="wrow_sb", bufs=2)
                    nc.scalar.copy(mwsb.rearrange("p nb c -> p (nb c)"), mw_ps)
                    num_all = acc.tile([128, NB, 66], F32, tag="num")
                    for bi in range(NB):
                        blk = b * NB + bi
                        tok = slice(C * blk, C * (blk + 1))
                        # ---- intra ----
                        sdt = psA.tile([128, 128], F32, tag="sdt", bufs=3)
                        nc.tensor.matmul(sdt, KpT[hp, tok], QpT[hp, tok],
                                         start=True, stop=True)
                        smdt = sm.tile([128, 128], F32R, tag="smdt")
                        nc.vector.tensor_mul(smdt, sdt, mwsb[:, bi, :])
                        nd = psA.tile([128, 66], F32, tag="np", bufs=3)
                        nc.tensor.matmul(nd, smdt, Vp1[:, blk, vc],
                                         start=True, stop=True)
                        nc.scalar.copy(num_all[:, bi, :], nd)
                        # ---- inter ----
                        for (beta, sz, g) in _decomp(bi):
                            pp = psA.tile([128, 66], F32, tag="np", bufs=3)
                            si = beta if sz == 1 else _COMB[(beta, sz)]
                            nc.tensor.matmul(pp, QpT[hp, tok], ST[hp, si, vc],
                                             start=True, stop=True)
                            sc = wt[:, blk, 12 * h + 7 + g:12 * h + 8 + g]
                            nc.vector.scalar_tensor_tensor(
                                out=num_all[:, bi, :], in0=pp, scalar=sc,
                                in1=num_all[:, bi, :],
                                op0=ALU.mult, op1=ALU.add)
                    # ---- divide (batched reciprocal) ----
                    dcol = smd.tile([128, NB], F32, tag="dcol")
                    nc.vector.tensor_copy(dcol, num_all[:, :, 64])
                    rec = smd.tile([128, NB], F32, tag="rec")
                    nc.vector.reciprocal(rec, dcol)
                    for bi in range(NB):
                        nc.vector.tensor_scalar_mul(
                            attn_a[:, b * NB + bi, 64 * h:64 * (h + 1)],
                            num_all[:, bi, 0:64], rec[:, bi:bi + 1])
                # ---- output projection for batch b (overlaps next batch) ----
                for q in range(4):
                    ot = sm.tile([128, 2, 1024], F32, tag="ostage", bufs=2)
                    for j in range(2):
                        blk = b * NB + 2 * q + j
                        att_ps = psA.tile([128, 128], F32, tag="sdt", bufs=3)
                        nc.tensor.transpose(att_ps, attn_a[:, blk, :], ident)
                        nc.vector.tensor_copy(attnT[:, C * blk:C * (blk + 1)],
                                              att_ps)
                        for half in range(2):
                            po = psA.tile([128, 512], F32, tag="np", bufs=3)
                            nc.tensor.matmul(
                                po, attnT[:, C * blk:C * (blk + 1)],
                                ow_sb[:, 512 * half:512 * (half + 1)],
                                start=True, stop=True)
                            dsts = ot[:, j, 512 * half:512 * (half + 1)]
                            if (j + half) % 2 == 0:
                                nc.scalar.copy(dsts, po)
                            else:
                                nc.vector.tensor_copy(dsts, po)
                    for j in range(2):
                        blk = b * NB + 2 * q + j
                        nc.sync.dma_start(
                            out=out_d[C * blk:C * (blk + 1), :],
                            in_=ot[:, j, :])

    nc.compile()
    return nc


def _host_prep(inputs):
    x = np.ascontiguousarray(np.asarray(inputs["x"], np.float32).reshape(B * T, DM))
    xT = np.ascontiguousarray(x.T)
    qw = np.asarray(inputs["qw"], np.float32)
    kw = np.asarray(inputs["kw"], np.float32)
    vw = np.asarray(inputs["vw"], np.float32)
    lw = np.asarray(inputs["lw"], np.float32)
    ow = np.asarray(inputs["ow"], np.float32)
    qb = np.asarray(inputs["qb"], np.float32)
    kb = np.asarray(inputs["kb"], np.float32)
    vb = np.asarray(inputs["vb"], np.float32)
    lb = np.asarray(inputs["lb"], np.float32)

    colind, rowind, replt = _build_slot_consts()
    m127_host = np.zeros((128, 1), np.uint8)
    m127_host[127, 0] = 1

    in_maps = []
    for c in range(NCORES):
        hA, hB = 2 * c, 2 * c + 1
        wallh = np.zeros((DM, WALLN), np.float32)
        wallh[:, 0:128] = qw[:, 128 * c:128 * (c + 1)]
        wallh[:, 128:256] = kw[:, 128 * c:128 * (c + 1)]
        wallh[:, 256 + 0:256 + 64] = vw[:, 128 * c:128 * c + 64]
        wallh[:, 256 + 66:256 + 130] = vw[:, 128 * c + 64:128 * (c + 1)]
        wallh[:, 256 + 132:256 + 260] = kw[:, 128 * c:128 * (c + 1)]
        wallh[:, 256 + 260:256 + 272] = lw[:, 12 * hA:12 * hA + 12]
        wallh[:, 256 + 272:256 + 284] = lw[:, 12 * hB:12 * hB + 12]
        cvrh = np.zeros((128, 1152), np.float32)
        cvrh[:, 0:128] = colind
        cvrh[:, 128:1152] = ow[128 * c:128 * (c + 1), :]
        cvfh = np.zeros((128, 1032), np.float32)
        cvfh[:, 0:1024] = np.tile(rowind, (1, NB))
        cvfh[127, 1028] = 1.0
        cvfh[:, 1024] = qb[128 * c:128 * (c + 1)]
        cvfh[:, 1025] = qb[128 * c:128 * (c + 1)] + 1.0
        cvfh[:, 1026] = kb[128 * c:128 * (c + 1)]
        cvfh[:, 1027] = kb[128 * c:128 * (c + 1)] + 1.0
        bias1h = np.zeros((128, 284), np.float32)
        bias1h[:, 0:64] = vb[128 * c:128 * c + 64]
        bias1h[:, 66:130] = vb[128 * c + 64:128 * (c + 1)]
        bias1h[:, 132:260] = kb[128 * c:128 * (c + 1)]
        bias1h[:, 260:272] = lb[12 * hA:12 * hA + 12]
        bias1h[:, 272:284] = lb[12 * hB:12 * hB + 12]
        in_maps.append({
            "xT": xT,
            "wall": np.ascontiguousarray(wallh),
            "cvr": cvrh,
            "cvf": cvfh,
            "replt": np.ascontiguousarray(replt),
            "m127": m127_host,
            "bias1": bias1h,
        })
    with_bias = bool(np.any(vb) or np.any(kb) or np.any(lb))
    return in_maps, with_bias


def kernel(**inputs) -> np.ndarray:
    in_maps, with_bias = _host_prep(inputs)
    if with_bias not in _PROGRAM_CACHE:
        _PROGRAM_CACHE[with_bias] = _build_program(with_bias)
    nc = _PROGRAM_CACHE[with_bias]
    res = run_bass_kernel_spmd(nc, in_maps, list(range(NCORES)))
    ob = np.asarray(inputs["ob"], np.float32)
    out = np.zeros((B * T, DM), np.float32)
    for r in res.results:
        out += np.asarray(r["out"], np.float32)
    out += ob[None, :]
    return out.reshape(B, T, DM)
